# revision 1
# baseline (speedup 1.0000x reference)
"""ContactAwareLoss Trainium2 kernel.

Strategy: pure data-parallel over batch (512 rows -> 8 cores x 64 rows).
Each core computes four partial sums over its shard:
  [0] sum_{t,h} probs2 * |dist - 0.1|            (contact distance, unnormalized)
  [1] sum_{j,h} probs2[j+1] * ||r[j+1]-r[j]||     (contact velocity, unnormalized)
  [2] sum_{t,h} first_contact * (5-tap sum of |second diff of dist|)
  [3] sum first_contact                           (count)
The host divides by the global element counts / count and applies the ramp.

On-chip layout: partition p = half*64 + b  (sequence halved so 64 batch rows
fill 128 partitions); free dim = time within the half, processed in W-wide
chunks with a 3-element halo on both sides.  The halo at the half boundary is
filled with real neighbour data via small extra DMAs; the halo at the global
sequence ends is zero-filled and the affected contributions are masked by
zeroing q/vd edge columns (smoothness valid t in [3, seq-3), velocity valid
j in [0, seq-1)).

Engine split:
 - DMA: hand+obj on the sync HWDGE ring, probs on the scalar HWDGE ring
   (both fp32 - SWDGE cast DMAs measured ~75 GB/s, far slower than fp32
   HWDGE, so the bf16 conversion rides the compute ops' output dtype).
 - DVE: r (fp32->bf16), c-sums, diffs/movsum (bf16 2x mode - all time shifts
   in the (t, h*c)-major layouts are 4-byte aligned), fused weighted-sum
   accumulators (scalar_tensor_tensor).
 - ScalarE: Square / Sqrt / Abs (contiguous APs only - strided activation
   outputs measured 5x slow).
 - GpSimd: first-contact mask pipeline (cb/fc+count) to offload the DVE.
"""

import numpy as np

BS, SEQ = 512, 4096
N_CORES = 8
W_FULL = 512  # chunk width (per half-sequence)


def build_nc(bs_local, seq, W):
    import concourse.bass as bass
    import concourse.bacc as bacc
    import concourse.tile as tile
    from concourse import mybir

    f32 = mybir.dt.float32
    bf16 = mybir.dt.bfloat16
    Alu = mybir.AluOpType
    Act = mybir.ActivationFunctionType

    P = 2 * bs_local          # partitions used
    HS = seq // 2             # timesteps per partition row
    assert HS % W == 0
    C = HS // W               # chunks
    E = W + 6                 # chunk width incl. +-3 halo
    H = P // 2

    nc = bacc.Bacc("TRN2", target_bir_lowering=False, debug=False)
    hand = nc.dram_tensor("pred_hand_pos", [bs_local, seq, 2, 3], f32, kind="ExternalInput")
    obj = nc.dram_tensor("pred_obj_pos", [bs_local, seq, 3], f32, kind="ExternalInput")
    probs = nc.dram_tensor("contact_probs", [bs_local, seq, 3], f32, kind="ExternalInput")
    partials = nc.dram_tensor("partials", [P, 4], f32, kind="ExternalOutput")

    def dram_ap(t, offset, dims):
        return bass.AP(tensor=t, offset=offset, ap=[list(d) for d in dims])

    with tile.TileContext(nc) as tc:
        import contextlib
        with contextlib.ExitStack() as ctx:
            inp = ctx.enter_context(tc.tile_pool(name="inp", bufs=2))
            work = ctx.enter_context(tc.tile_pool(name="work", bufs=1))
            singles = ctx.enter_context(tc.tile_pool(name="singles", bufs=1))

            l1s = singles.tile([P, C], f32)
            l2s = singles.tile([P, C], f32)
            sms = singles.tile([P, C], f32)
            cns = singles.tile([P, C], f32)
            outt = singles.tile([P, 4], f32)
            c_neg01 = singles.tile([P, 1], f32)
            nc.vector.memset(c_neg01[:], -0.1)

            for c in range(C):
                t0 = c * W  # first owned timestep (within half)
                t_lo = max(0, t0 - 3)
                t_hi = min(HS, t0 + W + 3)
                col_lo = t_lo - (t0 - 3)
                ncols = t_hi - t_lo

                hand_t = inp.tile([P, E, 6], f32)
                obj_t = inp.tile([P, E, 3], f32)
                probs_t = inp.tile([P, E, 3], f32)

                loads = (
                    (hand_t, hand, 6, nc.sync),
                    (obj_t, obj, 3, nc.sync),
                    (probs_t, probs, 3, nc.scalar),
                )
                for tile_buf, ten, k, eng in loads:
                    eng.dma_start(
                        out=tile_buf[:, col_lo:col_lo + ncols, :],
                        in_=dram_ap(ten, t_lo * k,
                                    [[HS * k, 2], [seq * k, bs_local], [1, ncols * k]]),
                    )
                    if c == 0:
                        eng.dma_start(
                            out=tile_buf[H:P, 0:3, :],
                            in_=dram_ap(ten, (HS - 3) * k,
                                        [[seq * k, bs_local], [1, 3 * k]]),
                        )
                        nc.vector.memset(tile_buf[0:H, 0:3, :], 0.0)
                    if c == C - 1:
                        eng.dma_start(
                            out=tile_buf[0:H, W + 3:E, :],
                            in_=dram_ap(ten, HS * k,
                                        [[seq * k, bs_local], [1, 3 * k]]),
                        )
                        nc.vector.memset(tile_buf[H:P, W + 3:E, :], 0.0)

                # ---- r = hand - obj (one strided sub per hand, fp32 -> bf16) ----
                r_t = work.tile([P, E, 6], bf16)
                for h in range(2):
                    nc.vector.tensor_sub(r_t[:, :, 3 * h:3 * h + 3],
                                         hand_t[:, :, 3 * h:3 * h + 3], obj_t[:])

                # ---- d2 = sum_c r^2 (Square on ACT, two strided adds) ----
                sq_t = work.tile([P, E, 6], bf16)
                nc.scalar.activation(sq_t[:], r_t[:], Act.Square)
                sqa = sq_t[:]

                def csum(dst, src_ap, n):
                    """dst[t,h] = src[t,3h]+src[t,3h+1]+src[t,3h+2] over n positions."""
                    v = [bass.AP(tensor=src_ap.tensor, offset=src_ap.offset + cc,
                                 ap=[src_ap.ap[0], [3, 2 * n]]) for cc in range(3)]
                    tmp = work.tile([P, n, 2], bf16, tag=f"csum_tmp")
                    ta = bass.AP(tensor=tmp.tensor, offset=tmp[:].offset,
                                 ap=[tmp[:].ap[0], [1, 2 * n]])
                    nc.vector.tensor_add(ta, v[0], v[1])
                    nc.vector.tensor_add(dst, ta, v[2])

                d2_t = work.tile([P, E, 2], bf16)
                csum(d2_t[:].opt(), sqa, E)
                d_t = work.tile([P, E, 2], bf16)
                nc.scalar.activation(d_t[:], d2_t[:], Act.Sqrt)

                # ---- contact distance partial ----
                derr_t = work.tile([P, W, 2], bf16)
                nc.scalar.activation(derr_t[:], d_t[:, 3:3 + W, :], Act.Abs, bias=c_neg01[:])
                l1p_t = work.tile([P, W, 2], f32)
                nc.vector.scalar_tensor_tensor(
                    out=l1p_t[:], in0=probs_t[:, 3:3 + W, 0:2], scalar=1.0, in1=derr_t[:],
                    op0=Alu.mult, op1=Alu.mult, accum_out=l1s[:, c:c + 1])

                # ---- velocity ----
                dr_t = work.tile([P, W, 6], bf16)
                nc.vector.tensor_sub(dr_t[:], r_t[:, 4:4 + W, :], r_t[:, 3:3 + W, :])
                dsq_t = work.tile([P, W, 6], bf16)
                nc.scalar.activation(dsq_t[:], dr_t[:], Act.Square)
                v2_t = work.tile([P, W, 2], bf16)
                csum(v2_t[:].opt(), dsq_t[:], W)
                vd_t = work.tile([P, W, 2], bf16)
                nc.scalar.activation(vd_t[:], v2_t[:], Act.Sqrt)
                if c == C - 1:
                    nc.vector.memset(vd_t[H:P, W - 1:W, :], 0.0)  # j=seq-1 invalid
                l2p_t = work.tile([P, W, 2], f32)
                nc.vector.scalar_tensor_tensor(
                    out=l2p_t[:], in0=probs_t[:, 4:4 + W, 0:2], scalar=1.0, in1=vd_t[:],
                    op0=Alu.mult, op1=Alu.mult, accum_out=l2s[:, c:c + 1])

                # ---- smoothness ----
                e_t = work.tile([P, E - 1, 2], bf16)
                nc.vector.tensor_sub(e_t[:], d_t[:, 1:E, :], d_t[:, 0:E - 1, :])
                sdp_t = work.tile([P, W + 4, 2], bf16)
                nc.vector.tensor_sub(sdp_t[:], e_t[:, 0:W + 4, :], e_t[:, 1:W + 5, :])
                sd_t = work.tile([P, W + 4, 2], bf16)
                nc.scalar.activation(sd_t[:], sdp_t[:], Act.Abs)
                s2_t = work.tile([P, W + 3, 2], bf16)
                nc.vector.tensor_add(s2_t[:], sd_t[:, 0:W + 3, :], sd_t[:, 1:W + 4, :])
                s4_t = work.tile([P, W + 1, 2], bf16)
                nc.vector.tensor_add(s4_t[:], s2_t[:, 0:W + 1, :], s2_t[:, 2:W + 3, :])
                sm5_t = work.tile([P, W, 2], bf16)
                nc.vector.tensor_add(sm5_t[:], s4_t[:, 0:W, :], sd_t[:, 4:W + 4, :])

                # ---- first contact mask + count (on GpSimd) ----
                cb_t = work.tile([P, W + 1, 2], bf16)
                nc.gpsimd.tensor_scalar(
                    out=cb_t[:], in0=probs_t[:, 2:3 + W, 0:2],
                    scalar1=0.5, scalar2=None, op0=Alu.is_gt)
                q_t = work.tile([P, W, 2], bf16)
                nc.gpsimd.tensor_sub(q_t[:], cb_t[:, 1:W + 1, :], cb_t[:, 0:W, :])
                if c == 0:
                    nc.vector.memset(q_t[0:H, 0:3, :], 0.0)  # t<3 (incl. forced-false t=0)
                if c == C - 1:
                    nc.vector.memset(q_t[H:P, W - 3:W, :], 0.0)  # t >= seq-3
                fc_t = work.tile([P, W, 2], bf16)
                nc.vector.tensor_scalar(
                    out=fc_t[:], in0=q_t[:], scalar1=0.0, scalar2=0.0,
                    op0=Alu.max, op1=Alu.add, accum_out=cns[:, c:c + 1])

                smp_t = work.tile([P, W, 2], f32)
                nc.vector.scalar_tensor_tensor(
                    out=smp_t[:], in0=sm5_t[:], scalar=1.0, in1=fc_t[:],
                    op0=Alu.mult, op1=Alu.mult, accum_out=sms[:, c:c + 1])

            # ---- final per-partition combine + store ----
            for i, slot in enumerate((l1s, l2s, sms, cns)):
                nc.vector.tensor_reduce(outt[:, i:i + 1], slot[:], axis=mybir.AxisListType.X, op=Alu.add)
            nc.sync.dma_start(out=partials.ap(), in_=outt[:])

    nc.compile()
    return nc


_cache = {}


def _get_nc(bs_local, seq, W):
    key = (bs_local, seq, W)
    if key not in _cache:
        _cache[key] = build_nc(bs_local, seq, W)
    return _cache[key]


def combine_partials(parts, bs, seq, training_step):
    """parts: float array [..., 4] of per-core/per-partition partial sums."""
    s = np.asarray(parts, dtype=np.float64).reshape(-1, 4).sum(axis=0)
    l1 = s[0] / (bs * seq * 2)
    l2 = s[1] / (bs * (seq - 1) * 2) if seq > 1 else 0.0
    cnt = s[3]
    sm = (s[2] / 5.0) / max(cnt, 1.0) if (seq > 5 and cnt > 0) else 0.0
    ramp = min(1.0, float(training_step) / 1000.0)
    return np.array(ramp * (1.0 * l1 + 0.5 * l2 + 0.3 * sm), dtype=np.float32)


def _run(pred_hand_pos, pred_obj_pos, contact_probs, **spmd_kwargs):
    from concourse.bass_utils import run_bass_kernel_spmd

    hand = np.ascontiguousarray(np.asarray(pred_hand_pos, dtype=np.float32))
    obj = np.ascontiguousarray(np.asarray(pred_obj_pos, dtype=np.float32))
    probs = np.ascontiguousarray(np.asarray(contact_probs, dtype=np.float32))
    bs, seq = hand.shape[:2]
    bs_local = bs // N_CORES
    nc = _get_nc(bs_local, seq, W_FULL)

    in_maps = []
    for i in range(N_CORES):
        sl = slice(i * bs_local, (i + 1) * bs_local)
        in_maps.append({
            "pred_hand_pos": hand[sl],
            "pred_obj_pos": obj[sl],
            "contact_probs": probs[sl],
        })
    # The axon terminal occasionally reports the exec unit unrecoverable on
    # the first touch after a previous process's teardown; a retry lands on a
    # recovered device.
    last_err = None
    for _ in range(3):
        try:
            res = run_bass_kernel_spmd(
                nc, in_maps, core_ids=list(range(N_CORES)), **spmd_kwargs
            )
            parts = np.stack([res.results[i]["partials"] for i in range(N_CORES)])
            return parts, res
        except Exception as e:  # noqa: BLE001
            last_err = e
    raise last_err


def kernel(pred_hand_pos, pred_obj_pos, contact_probs, training_step):
    bs, seq = np.asarray(pred_hand_pos).shape[:2]
    parts, _ = _run(pred_hand_pos, pred_obj_pos, contact_probs)
    return combine_partials(parts, bs, seq, training_step)



# revision 2
# speedup vs baseline: 1.7686x; 1.7686x over previous
"""ContactAwareLoss Trainium2 kernel.

Strategy: pure data-parallel over batch (512 rows -> 8 cores x 64 rows).
Each core computes four partial sums over its shard:
  [0] sum_{t,h} probs2 * |dist - 0.1|            (contact distance, unnormalized)
  [1] sum_{j,h} probs2[j+1] * ||r[j+1]-r[j]||     (contact velocity, unnormalized)
  [2] sum_{t,h} first_contact * (5-tap sum of |second diff of dist|)
  [3] sum first_contact                           (count)
The host divides by the global element counts / count and applies the ramp.

On-chip layout: partition p = half*64 + b  (sequence halved so 64 batch rows
fill 128 partitions); free dim = time within the half, processed in W-wide
chunks with a 3-element halo on both sides.  The halo at the half boundary is
filled with real neighbour data via small extra DMAs; the halo at the global
sequence ends is zero-filled and the affected contributions are masked by
zeroing q/vd edge columns (smoothness valid t in [3, seq-3), velocity valid
j in [0, seq-1)).

Engine split:
 - DMA: hand+obj on the sync HWDGE ring, probs on the scalar HWDGE ring
   (both fp32 - SWDGE cast DMAs measured ~75 GB/s, far slower than fp32
   HWDGE, so the bf16 conversion rides the compute ops' output dtype).
 - DVE: r (fp32->bf16), c-sums, diffs/movsum (bf16 2x mode - all time shifts
   in the (t, h*c)-major layouts are 4-byte aligned), fused weighted-sum
   accumulators (scalar_tensor_tensor).
 - ScalarE: Square / Sqrt / Abs (contiguous APs only - strided activation
   outputs measured 5x slow).
 - GpSimd: first-contact mask pipeline (cb/fc+count) to offload the DVE.
"""

import numpy as np

BS, SEQ = 512, 4096
N_CORES = 8
W_FULL = 512  # chunk width (per half-sequence)


def build_nc(bs_local, seq, W):
    import concourse.bass as bass
    import concourse.bacc as bacc
    import concourse.tile as tile
    from concourse import mybir

    f32 = mybir.dt.float32
    bf16 = mybir.dt.bfloat16
    Alu = mybir.AluOpType
    Act = mybir.ActivationFunctionType

    P = 2 * bs_local          # partitions used
    HS = seq // 2             # timesteps per partition row
    assert HS % W == 0
    C = HS // W               # chunks
    E = W + 6                 # chunk width incl. +-3 halo
    H = P // 2

    nc = bacc.Bacc("TRN2", target_bir_lowering=False, debug=False)
    hand = nc.dram_tensor("pred_hand_pos", [bs_local, seq, 2, 3], f32, kind="ExternalInput")
    obj = nc.dram_tensor("pred_obj_pos", [bs_local, seq, 3], f32, kind="ExternalInput")
    probs = nc.dram_tensor("contact_probs", [bs_local, seq, 3], f32, kind="ExternalInput")
    partials = nc.dram_tensor("partials", [P, 4], f32, kind="ExternalOutput")

    def dram_ap(t, offset, dims):
        return bass.AP(tensor=t, offset=offset, ap=[list(d) for d in dims])

    with tile.TileContext(nc) as tc:
        import contextlib
        with contextlib.ExitStack() as ctx:
            inp = ctx.enter_context(tc.tile_pool(name="inp", bufs=2))
            work = ctx.enter_context(tc.tile_pool(name="work", bufs=1))
            singles = ctx.enter_context(tc.tile_pool(name="singles", bufs=1))

            l1s = singles.tile([P, C], f32)
            l2s = singles.tile([P, C], f32)
            sms = singles.tile([P, C], f32)
            cns = singles.tile([P, C], f32)
            outt = singles.tile([P, 4], f32)
            c_neg01 = singles.tile([P, 1], f32)
            nc.vector.memset(c_neg01[:], -0.1)

            for c in range(C):
                t0 = c * W  # first owned timestep (within half)
                t_lo = max(0, t0 - 3)
                t_hi = min(HS, t0 + W + 3)
                col_lo = t_lo - (t0 - 3)
                ncols = t_hi - t_lo

                hand_t = inp.tile([P, E, 6], f32)
                obj_t = inp.tile([P, E, 3], f32)
                probs_t = inp.tile([P, E, 3], f32)

                loads = (
                    (hand_t, hand, 6, nc.sync),
                    (obj_t, obj, 3, nc.scalar),
                    (probs_t, probs, 3, nc.scalar),
                )
                for tile_buf, ten, k, eng in loads:
                    # One dma_start per sequence-half: the descriptor->SDMA-engine
                    # assignment follows the outermost AP dim, so an outer dim of
                    # bs_local (64) spreads across all 16 engines instead of 2.
                    for h in range(2):
                        eng.dma_start(
                            out=tile_buf[h * H:(h + 1) * H, col_lo:col_lo + ncols, :],
                            in_=dram_ap(ten, (h * HS + t_lo) * k,
                                        [[seq * k, bs_local], [1, ncols * k]]),
                        )
                    if c == 0:
                        eng.dma_start(
                            out=tile_buf[H:P, 0:3, :],
                            in_=dram_ap(ten, (HS - 3) * k,
                                        [[seq * k, bs_local], [1, 3 * k]]),
                        )
                        nc.vector.memset(tile_buf[0:H, 0:3, :], 0.0)
                    if c == C - 1:
                        eng.dma_start(
                            out=tile_buf[0:H, W + 3:E, :],
                            in_=dram_ap(ten, HS * k,
                                        [[seq * k, bs_local], [1, 3 * k]]),
                        )
                        nc.vector.memset(tile_buf[H:P, W + 3:E, :], 0.0)

                # ---- r = hand - obj (one strided sub per hand, fp32 -> bf16) ----
                r_t = work.tile([P, E, 6], bf16)
                for h in range(2):
                    nc.vector.tensor_sub(r_t[:, :, 3 * h:3 * h + 3],
                                         hand_t[:, :, 3 * h:3 * h + 3], obj_t[:])

                # ---- d2 = sum_c r^2 (Square on ACT, two strided adds) ----
                sq_t = work.tile([P, E, 6], bf16)
                nc.scalar.activation(sq_t[:], r_t[:], Act.Square)
                sqa = sq_t[:]

                def csum(dst, src_ap, n):
                    """dst[t,h] = src[t,3h]+src[t,3h+1]+src[t,3h+2] over n positions."""
                    v = [bass.AP(tensor=src_ap.tensor, offset=src_ap.offset + cc,
                                 ap=[src_ap.ap[0], [3, 2 * n]]) for cc in range(3)]
                    tmp = work.tile([P, n, 2], bf16, tag=f"csum_tmp")
                    ta = bass.AP(tensor=tmp.tensor, offset=tmp[:].offset,
                                 ap=[tmp[:].ap[0], [1, 2 * n]])
                    nc.vector.tensor_add(ta, v[0], v[1])
                    nc.vector.tensor_add(dst, ta, v[2])

                d2_t = work.tile([P, E, 2], bf16)
                csum(d2_t[:].opt(), sqa, E)
                d_t = work.tile([P, E, 2], bf16)
                nc.scalar.activation(d_t[:], d2_t[:], Act.Sqrt)

                # ---- contact distance partial ----
                derr_t = work.tile([P, W, 2], bf16)
                nc.scalar.activation(derr_t[:], d_t[:, 3:3 + W, :], Act.Abs, bias=c_neg01[:])
                l1p_t = work.tile([P, W, 2], f32)
                nc.vector.scalar_tensor_tensor(
                    out=l1p_t[:], in0=probs_t[:, 3:3 + W, 0:2], scalar=1.0, in1=derr_t[:],
                    op0=Alu.mult, op1=Alu.mult, accum_out=l1s[:, c:c + 1])

                # ---- velocity ----
                dr_t = work.tile([P, W, 6], bf16)
                nc.vector.tensor_sub(dr_t[:], r_t[:, 4:4 + W, :], r_t[:, 3:3 + W, :])
                dsq_t = work.tile([P, W, 6], bf16)
                nc.scalar.activation(dsq_t[:], dr_t[:], Act.Square)
                v2_t = work.tile([P, W, 2], bf16)
                csum(v2_t[:].opt(), dsq_t[:], W)
                vd_t = work.tile([P, W, 2], bf16)
                nc.scalar.activation(vd_t[:], v2_t[:], Act.Sqrt)
                if c == C - 1:
                    nc.vector.memset(vd_t[H:P, W - 1:W, :], 0.0)  # j=seq-1 invalid
                l2p_t = work.tile([P, W, 2], f32)
                nc.vector.scalar_tensor_tensor(
                    out=l2p_t[:], in0=probs_t[:, 4:4 + W, 0:2], scalar=1.0, in1=vd_t[:],
                    op0=Alu.mult, op1=Alu.mult, accum_out=l2s[:, c:c + 1])

                # ---- smoothness ----
                e_t = work.tile([P, E - 1, 2], bf16)
                nc.vector.tensor_sub(e_t[:], d_t[:, 1:E, :], d_t[:, 0:E - 1, :])
                sdp_t = work.tile([P, W + 4, 2], bf16)
                nc.vector.tensor_sub(sdp_t[:], e_t[:, 0:W + 4, :], e_t[:, 1:W + 5, :])
                sd_t = work.tile([P, W + 4, 2], bf16)
                nc.scalar.activation(sd_t[:], sdp_t[:], Act.Abs)
                s2_t = work.tile([P, W + 3, 2], bf16)
                nc.vector.tensor_add(s2_t[:], sd_t[:, 0:W + 3, :], sd_t[:, 1:W + 4, :])
                s4_t = work.tile([P, W + 1, 2], bf16)
                nc.vector.tensor_add(s4_t[:], s2_t[:, 0:W + 1, :], s2_t[:, 2:W + 3, :])
                sm5_t = work.tile([P, W, 2], bf16)
                nc.vector.tensor_add(sm5_t[:], s4_t[:, 0:W, :], sd_t[:, 4:W + 4, :])

                # ---- first contact mask + count (on GpSimd) ----
                cb_t = work.tile([P, W + 1, 2], bf16)
                nc.gpsimd.tensor_scalar(
                    out=cb_t[:], in0=probs_t[:, 2:3 + W, 0:2],
                    scalar1=0.5, scalar2=None, op0=Alu.is_gt)
                q_t = work.tile([P, W, 2], bf16)
                nc.gpsimd.tensor_sub(q_t[:], cb_t[:, 1:W + 1, :], cb_t[:, 0:W, :])
                if c == 0:
                    nc.vector.memset(q_t[0:H, 0:3, :], 0.0)  # t<3 (incl. forced-false t=0)
                if c == C - 1:
                    nc.vector.memset(q_t[H:P, W - 3:W, :], 0.0)  # t >= seq-3
                fc_t = work.tile([P, W, 2], bf16)
                nc.vector.tensor_scalar(
                    out=fc_t[:], in0=q_t[:], scalar1=0.0, scalar2=0.0,
                    op0=Alu.max, op1=Alu.add, accum_out=cns[:, c:c + 1])

                smp_t = work.tile([P, W, 2], f32)
                nc.vector.scalar_tensor_tensor(
                    out=smp_t[:], in0=sm5_t[:], scalar=1.0, in1=fc_t[:],
                    op0=Alu.mult, op1=Alu.mult, accum_out=sms[:, c:c + 1])

            # ---- final per-partition combine + store ----
            for i, slot in enumerate((l1s, l2s, sms, cns)):
                nc.vector.tensor_reduce(outt[:, i:i + 1], slot[:], axis=mybir.AxisListType.X, op=Alu.add)
            nc.sync.dma_start(out=partials.ap(), in_=outt[:])

    nc.compile()
    return nc


_cache = {}


def _get_nc(bs_local, seq, W):
    key = (bs_local, seq, W)
    if key not in _cache:
        _cache[key] = build_nc(bs_local, seq, W)
    return _cache[key]


def combine_partials(parts, bs, seq, training_step):
    """parts: float array [..., 4] of per-core/per-partition partial sums."""
    s = np.asarray(parts, dtype=np.float64).reshape(-1, 4).sum(axis=0)
    l1 = s[0] / (bs * seq * 2)
    l2 = s[1] / (bs * (seq - 1) * 2) if seq > 1 else 0.0
    cnt = s[3]
    sm = (s[2] / 5.0) / max(cnt, 1.0) if (seq > 5 and cnt > 0) else 0.0
    ramp = min(1.0, float(training_step) / 1000.0)
    return np.array(ramp * (1.0 * l1 + 0.5 * l2 + 0.3 * sm), dtype=np.float32)


def _run(pred_hand_pos, pred_obj_pos, contact_probs, **spmd_kwargs):
    from concourse.bass_utils import run_bass_kernel_spmd

    hand = np.ascontiguousarray(np.asarray(pred_hand_pos, dtype=np.float32))
    obj = np.ascontiguousarray(np.asarray(pred_obj_pos, dtype=np.float32))
    probs = np.ascontiguousarray(np.asarray(contact_probs, dtype=np.float32))
    bs, seq = hand.shape[:2]
    bs_local = bs // N_CORES
    nc = _get_nc(bs_local, seq, W_FULL)

    in_maps = []
    for i in range(N_CORES):
        sl = slice(i * bs_local, (i + 1) * bs_local)
        in_maps.append({
            "pred_hand_pos": hand[sl],
            "pred_obj_pos": obj[sl],
            "contact_probs": probs[sl],
        })
    # The axon terminal occasionally reports the exec unit unrecoverable on
    # the first touch after a previous process's teardown; a retry lands on a
    # recovered device.
    last_err = None
    for _ in range(3):
        try:
            res = run_bass_kernel_spmd(
                nc, in_maps, core_ids=list(range(N_CORES)), **spmd_kwargs
            )
            parts = np.stack([res.results[i]["partials"] for i in range(N_CORES)])
            return parts, res
        except Exception as e:  # noqa: BLE001
            last_err = e
    raise last_err


def kernel(pred_hand_pos, pred_obj_pos, contact_probs, training_step):
    bs, seq = np.asarray(pred_hand_pos).shape[:2]
    parts, _ = _run(pred_hand_pos, pred_obj_pos, contact_probs)
    return combine_partials(parts, bs, seq, training_step)



# revision 4
# speedup vs baseline: 2.6174x; 1.4799x over previous
"""ContactAwareLoss Trainium2 kernel.

Strategy: pure data-parallel over batch (512 rows -> 8 cores x 64 rows).
Each core computes four partial sums over its shard:
  [0] sum_{t,h} probs2 * |dist - 0.1|            (contact distance, unnormalized)
  [1] sum_{j,h} probs2[j+1] * ||r[j+1]-r[j]||     (contact velocity, unnormalized)
  [2] sum_{t,h} first_contact * (5-tap sum of |second diff of dist|)
  [3] sum first_contact                           (count)
The host divides by the global element counts / count and applies the ramp.

On-chip layout: partition p = half*64 + b  (sequence halved so 64 batch rows
fill 128 partitions); free dim = time within the half, processed in W-wide
chunks with a 3-element halo on both sides.  The halo at the half boundary is
filled with real neighbour data via small extra DMAs; the halo at the global
sequence ends is zero-filled and the affected contributions are masked by
zeroing q/vd edge columns (smoothness valid t in [3, seq-3), velocity valid
j in [0, seq-1)).

DMA: one dma_start per tensor per sequence-half so the outermost AP dim is
bs_local (64) -- the descriptor->SDMA-engine assignment follows the outermost
AP dim, so this spreads the load over all 16 engines instead of 2.

Compute layout: the hot geometry runs in a (c, t, h) free-dim layout -- three
channel blocks, each [t, 2 hands] -- produced by ONE fused fp32 subtract that
deinterleaves hand (t,h,c) and broadcasts obj via a stride-0 AP while casting
to bf16.  Channel sums (d2, v2) are then fully contiguous bf16 adds (DVE 2x
packed mode), and every time-shift is a whole (h0,h1) pair = 4 bytes, keeping
all smoothness/velocity shifts 4B-aligned for 2x.  Square/Sqrt/Abs ride the
scalar (ACT) engine with contiguous APs; the weight ops use tensor_scalar /
scalar_tensor_tensor with fp32 accumulators (scalar operands are exempt from
the 2x dtype rule).  GpSimd is not used (measured ~10x below its cost model).
"""

import numpy as np

BS, SEQ = 512, 4096
N_CORES = 8
W_FULL = 512  # chunk width (per half-sequence)


def build_nc(bs_local, seq, W):
    import concourse.bass as bass
    import concourse.bacc as bacc
    import concourse.tile as tile
    from concourse import mybir

    f32 = mybir.dt.float32
    bf16 = mybir.dt.bfloat16
    Alu = mybir.AluOpType
    Act = mybir.ActivationFunctionType

    P = 2 * bs_local          # partitions used
    HS = seq // 2             # timesteps per partition row
    assert HS % W == 0
    C = HS // W               # chunks
    E = W + 6                 # chunk width incl. +-3 halo
    H = P // 2

    nc = bacc.Bacc("TRN2", target_bir_lowering=False, debug=False)
    hand = nc.dram_tensor("pred_hand_pos", [bs_local, seq, 2, 3], f32, kind="ExternalInput")
    obj = nc.dram_tensor("pred_obj_pos", [bs_local, seq, 3], f32, kind="ExternalInput")
    probs = nc.dram_tensor("contact_probs", [bs_local, seq, 3], f32, kind="ExternalInput")
    partials = nc.dram_tensor("partials", [P, 4], f32, kind="ExternalOutput")

    def dram_ap(t, offset, dims):
        return bass.AP(tensor=t, offset=offset, ap=[list(d) for d in dims])

    with tile.TileContext(nc) as tc:
        import contextlib
        with contextlib.ExitStack() as ctx:
            inp = ctx.enter_context(tc.tile_pool(name="inp", bufs=2))
            work = ctx.enter_context(tc.tile_pool(name="work", bufs=2))
            singles = ctx.enter_context(tc.tile_pool(name="singles", bufs=1))

            l1s = singles.tile([P, C], f32)
            l2s = singles.tile([P, C], f32)
            sms = singles.tile([P, C], f32)
            cns = singles.tile([P, C], f32)
            outt = singles.tile([P, 4], f32)
            c_neg01 = singles.tile([P, 1], f32)
            nc.vector.memset(c_neg01[:], -0.1)

            for c in range(C):
                t0 = c * W  # first owned timestep (within half)
                t_lo = max(0, t0 - 3)
                t_hi = min(HS, t0 + W + 3)
                col_lo = t_lo - (t0 - 3)
                ncols = t_hi - t_lo

                hand_t = inp.tile([P, E, 2, 3], f32)
                obj_t = inp.tile([P, E, 3], f32)
                probs_t = inp.tile([P, E, 3], f32)

                loads = (
                    (hand_t, hand, 6, nc.sync),
                    (obj_t, obj, 3, nc.scalar),
                    (probs_t, probs, 3, nc.scalar),
                )
                for tile_buf, ten, k, eng in loads:
                    for h in range(2):
                        eng.dma_start(
                            out=tile_buf[h * H:(h + 1) * H, col_lo:col_lo + ncols],
                            in_=dram_ap(ten, (h * HS + t_lo) * k,
                                        [[seq * k, bs_local], [1, ncols * k]]),
                        )
                    if c == 0:
                        eng.dma_start(
                            out=tile_buf[H:P, 0:3],
                            in_=dram_ap(ten, (HS - 3) * k,
                                        [[seq * k, bs_local], [1, 3 * k]]),
                        )
                        nc.vector.memset(tile_buf[0:H, 0:3], 0.0)
                    if c == C - 1:
                        eng.dma_start(
                            out=tile_buf[0:H, W + 3:E],
                            in_=dram_ap(ten, HS * k,
                                        [[seq * k, bs_local], [1, 3 * k]]),
                        )
                        nc.vector.memset(tile_buf[H:P, W + 3:E], 0.0)

                # ---- fused: r[c,t,h] = hand[t,h,c] - obj[t,c]  (fp32 -> bf16) ----
                # Single 1x fp32 op that deinterleaves hand and broadcasts obj.
                r_t = work.tile([P, 3, E, 2], bf16)
                hand_v = hand_t[:].transpose([0, 3, 1, 2])          # [P, 3, E, 2]
                obj_v = obj_t[:].transpose([0, 2, 1]).unsqueeze(3).broadcast_to([P, 3, E, 2])
                nc.vector.tensor_sub(r_t[:], hand_v, obj_v)

                # ---- sq = r^2 (ACT, contiguous) ----
                sq_t = work.tile([P, 3, E, 2], bf16)
                nc.scalar.activation(sq_t[:], r_t[:], Act.Square)

                # ---- d2 = sum_c sq  (two contiguous bf16 2x adds) ----
                d2a_t = work.tile([P, E, 2], bf16)
                nc.vector.tensor_add(d2a_t[:], sq_t[:, 0], sq_t[:, 1])
                d2_t = work.tile([P, E, 2], bf16)
                nc.vector.tensor_add(d2_t[:], d2a_t[:], sq_t[:, 2])
                d_t = work.tile([P, E, 2], bf16)
                nc.scalar.activation(d_t[:], d2_t[:], Act.Sqrt)

                # ---- probs2 -> bf16 once (strided fp32 read, 2x_2p copy) ----
                probsb_t = work.tile([P, E, 2], bf16)
                nc.vector.tensor_copy(probsb_t[:], probs_t[:, :, 0:2])

                # ---- contact distance partial ----
                derr_t = work.tile([P, W, 2], bf16)
                nc.scalar.activation(derr_t[:], d_t[:, 3:3 + W, :], Act.Abs, bias=c_neg01[:])
                l1p_t = work.tile([P, W, 2], bf16)
                nc.vector.scalar_tensor_tensor(
                    out=l1p_t[:], in0=probsb_t[:, 3:3 + W, :], scalar=1.0, in1=derr_t[:],
                    op0=Alu.mult, op1=Alu.mult, accum_out=l1s[:, c:c + 1])

                # ---- velocity ----
                dr_t = work.tile([P, 3, W, 2], bf16)
                nc.vector.tensor_sub(dr_t[:], r_t[:, :, 4:4 + W, :], r_t[:, :, 3:3 + W, :])
                dsq_t = work.tile([P, 3, W, 2], bf16)
                nc.scalar.activation(dsq_t[:], dr_t[:], Act.Square)
                v2a_t = work.tile([P, W, 2], bf16)
                nc.vector.tensor_add(v2a_t[:], dsq_t[:, 0], dsq_t[:, 1])
                v2_t = work.tile([P, W, 2], bf16)
                nc.vector.tensor_add(v2_t[:], v2a_t[:], dsq_t[:, 2])
                vd_t = work.tile([P, W, 2], bf16)
                nc.scalar.activation(vd_t[:], v2_t[:], Act.Sqrt)
                if c == C - 1:
                    nc.vector.memset(vd_t[H:P, W - 1:W, :], 0.0)  # j=seq-1 invalid
                l2p_t = work.tile([P, W, 2], bf16)
                nc.vector.scalar_tensor_tensor(
                    out=l2p_t[:], in0=probsb_t[:, 4:4 + W, :], scalar=1.0, in1=vd_t[:],
                    op0=Alu.mult, op1=Alu.mult, accum_out=l2s[:, c:c + 1])

                # ---- smoothness ----
                e_t = work.tile([P, E - 1, 2], bf16)
                nc.vector.tensor_sub(e_t[:], d_t[:, 1:E, :], d_t[:, 0:E - 1, :])
                sdp_t = work.tile([P, W + 4, 2], bf16)
                nc.vector.tensor_sub(sdp_t[:], e_t[:, 0:W + 4, :], e_t[:, 1:W + 5, :])
                sd_t = work.tile([P, W + 4, 2], bf16)
                nc.scalar.activation(sd_t[:], sdp_t[:], Act.Abs)
                s2_t = work.tile([P, W + 3, 2], bf16)
                nc.vector.tensor_add(s2_t[:], sd_t[:, 0:W + 3, :], sd_t[:, 1:W + 4, :])
                s4_t = work.tile([P, W + 1, 2], bf16)
                nc.vector.tensor_add(s4_t[:], s2_t[:, 0:W + 1, :], s2_t[:, 2:W + 3, :])
                sm5_t = work.tile([P, W, 2], bf16)
                nc.vector.tensor_add(sm5_t[:], s4_t[:, 0:W, :], sd_t[:, 4:W + 4, :])

                # ---- first contact mask + count (exact fp32 threshold) ----
                cb_t = work.tile([P, W + 1, 2], bf16)
                nc.vector.tensor_scalar(
                    out=cb_t[:], in0=probs_t[:, 2:3 + W, 0:2],
                    scalar1=0.5, scalar2=None, op0=Alu.is_gt)
                q_t = work.tile([P, W, 2], bf16)
                nc.vector.tensor_sub(q_t[:], cb_t[:, 1:W + 1, :], cb_t[:, 0:W, :])
                if c == 0:
                    nc.vector.memset(q_t[0:H, 0:3, :], 0.0)  # t<3 (incl. forced-false t=0)
                if c == C - 1:
                    nc.vector.memset(q_t[H:P, W - 3:W, :], 0.0)  # t >= seq-3
                fc_t = work.tile([P, W, 2], bf16)
                nc.vector.tensor_scalar(
                    out=fc_t[:], in0=q_t[:], scalar1=0.0, scalar2=0.0,
                    op0=Alu.max, op1=Alu.add, accum_out=cns[:, c:c + 1])

                smp_t = work.tile([P, W, 2], bf16)
                nc.vector.scalar_tensor_tensor(
                    out=smp_t[:], in0=sm5_t[:], scalar=1.0, in1=fc_t[:],
                    op0=Alu.mult, op1=Alu.mult, accum_out=sms[:, c:c + 1])

            # ---- final per-partition combine + store ----
            for i, slot in enumerate((l1s, l2s, sms, cns)):
                nc.vector.tensor_reduce(outt[:, i:i + 1], slot[:], axis=mybir.AxisListType.X, op=Alu.add)
            nc.sync.dma_start(out=partials.ap(), in_=outt[:])

    nc.compile()
    return nc


_cache = {}


def _get_nc(bs_local, seq, W):
    key = (bs_local, seq, W)
    if key not in _cache:
        _cache[key] = build_nc(bs_local, seq, W)
    return _cache[key]


def combine_partials(parts, bs, seq, training_step):
    """parts: float array [..., 4] of per-core/per-partition partial sums."""
    s = np.asarray(parts, dtype=np.float64).reshape(-1, 4).sum(axis=0)
    l1 = s[0] / (bs * seq * 2)
    l2 = s[1] / (bs * (seq - 1) * 2) if seq > 1 else 0.0
    cnt = s[3]
    sm = (s[2] / 5.0) / max(cnt, 1.0) if (seq > 5 and cnt > 0) else 0.0
    ramp = min(1.0, float(training_step) / 1000.0)
    return np.array(ramp * (1.0 * l1 + 0.5 * l2 + 0.3 * sm), dtype=np.float32)


def _run(pred_hand_pos, pred_obj_pos, contact_probs, **spmd_kwargs):
    from concourse.bass_utils import run_bass_kernel_spmd

    hand = np.ascontiguousarray(np.asarray(pred_hand_pos, dtype=np.float32))
    obj = np.ascontiguousarray(np.asarray(pred_obj_pos, dtype=np.float32))
    probs = np.ascontiguousarray(np.asarray(contact_probs, dtype=np.float32))
    bs, seq = hand.shape[:2]
    bs_local = bs // N_CORES
    nc = _get_nc(bs_local, seq, W_FULL)

    in_maps = []
    for i in range(N_CORES):
        sl = slice(i * bs_local, (i + 1) * bs_local)
        in_maps.append({
            "pred_hand_pos": hand[sl],
            "pred_obj_pos": obj[sl],
            "contact_probs": probs[sl],
        })
    # The axon terminal occasionally reports the exec unit unrecoverable on
    # the first touch after a previous process's teardown; a retry lands on a
    # recovered device.
    last_err = None
    for _ in range(3):
        try:
            res = run_bass_kernel_spmd(
                nc, in_maps, core_ids=list(range(N_CORES)), **spmd_kwargs
            )
            parts = np.stack([res.results[i]["partials"] for i in range(N_CORES)])
            return parts, res
        except Exception as e:  # noqa: BLE001
            last_err = e
    raise last_err


def kernel(pred_hand_pos, pred_obj_pos, contact_probs, training_step):
    bs, seq = np.asarray(pred_hand_pos).shape[:2]
    parts, _ = _run(pred_hand_pos, pred_obj_pos, contact_probs)
    return combine_partials(parts, bs, seq, training_step)


# revision 5
# speedup vs baseline: 2.7565x; 1.0532x over previous
"""ContactAwareLoss Trainium2 kernel.

Strategy: pure data-parallel over batch (512 rows -> 8 cores x 64 rows).
Each core computes four partial sums over its shard:
  [0] sum_{t,h} probs2 * |dist - 0.1|            (contact distance, unnormalized)
  [1] sum_{j,h} probs2[j+1] * ||r[j+1]-r[j]||     (contact velocity, unnormalized)
  [2] 2 * sum first_contact * (5-tap sum of |second diff of dist|)
  [3] 2 * sum first_contact                       (count; x2 from the +-1 Sign encoding)
The host divides by the global element counts / count and applies the ramp.

On-chip layout: partition p = half*64 + b  (sequence halved so 64 batch rows
fill 128 partitions); free dim = time within the half, processed in chunks
with a 3-element halo on both sides.  The first chunk is narrow so the
pipeline-fill DMA is short.  The halo at the half boundary is filled with
real neighbour data via small extra DMAs; the halo at the global sequence
ends is zero-filled and the affected contributions are masked by zeroing
q/vd edge columns.

DMA: one dma_start per tensor per sequence-half so the outermost AP dim is
bs_local (64) -- the descriptor->SDMA-engine assignment follows the outermost
AP dim, so this spreads the load over all 16 engines instead of 2.

Engine split (measured modes):
 - DVE: all tensor_tensor work in bf16 contiguous (2x packed mode); the
   geometry runs in a (c, t, h) layout so channel sums are contiguous adds;
   r is ONE fused fp32 subtract that deinterleaves hand and broadcasts obj
   via a stride-0 AP (1x, but replaces three ops).  fc = max(q,0) uses
   tensor_scalar WITHOUT accum_out (accum forces 1x; no-accum runs 4x).
 - ACT: Square/Sqrt/Abs chains + Sign for the contact threshold (strided
   fp32 inputs run at full ACT speed; exact fp32 compare, +-1 encoding).
 - TensorE (idle otherwise): all four row-sum reductions as ones-vector
   matmuls accumulating into PSUM across chunks (host only needs global
   sums, not per-partition ones).
 - GpSimd: unused (measured ~10x below its cost model).
"""

import numpy as np

BS, SEQ = 512, 4096
N_CORES = 8
CHUNKS = (128, 512, 512, 512, 384)  # sums to 2048 = HS; narrow first chunk


def build_nc(bs_local, seq, chunks):
    import concourse.bass as bass
    import concourse.bacc as bacc
    import concourse.tile as tile
    from concourse import mybir

    f32 = mybir.dt.float32
    bf16 = mybir.dt.bfloat16
    Alu = mybir.AluOpType
    Act = mybir.ActivationFunctionType

    P = 2 * bs_local          # partitions used
    HS = seq // 2             # timesteps per partition row
    assert sum(chunks) == HS
    C = len(chunks)
    H = P // 2

    nc = bacc.Bacc("TRN2", target_bir_lowering=False, debug=False)
    hand = nc.dram_tensor("pred_hand_pos", [bs_local, seq, 2, 3], f32, kind="ExternalInput")
    obj = nc.dram_tensor("pred_obj_pos", [bs_local, seq, 3], f32, kind="ExternalInput")
    probs = nc.dram_tensor("contact_probs", [bs_local, seq, 3], f32, kind="ExternalInput")
    partials = nc.dram_tensor("partials", [P, 4], f32, kind="ExternalOutput")

    def dram_ap(t, offset, dims):
        return bass.AP(tensor=t, offset=offset, ap=[list(d) for d in dims])

    def fview(t, dims):
        """Free-dim view of a tile with explicit [stride, count] dims."""
        return bass.AP(tensor=t.tensor, offset=t[:].offset,
                       ap=[t[:].ap[0]] + [list(d) for d in dims])

    with tile.TileContext(nc) as tc:
        import contextlib
        with contextlib.ExitStack() as ctx:
            inp = ctx.enter_context(tc.tile_pool(name="inp", bufs=2))
            work = ctx.enter_context(tc.tile_pool(name="work", bufs=2))
            singles = ctx.enter_context(tc.tile_pool(name="singles", bufs=1))
            psum = ctx.enter_context(tc.tile_pool(name="psum", bufs=1, space="PSUM"))

            outt = singles.tile([P, 4], f32)
            nc.vector.memset(outt[:], 0.0)
            c_neg01 = singles.tile([P, 1], f32)
            nc.vector.memset(c_neg01[:], -0.1)
            c_neg05 = singles.tile([P, 1], f32)
            nc.vector.memset(c_neg05[:], -0.5)
            ones = singles.tile([P, 1], bf16)
            nc.vector.memset(ones[:], 1.0)

            # PSUM accumulators: one bank each for l1/l2/sm/cnt column sums.
            accs = [psum.tile([1, 512], f32, name=f"acc{i}", tag=f"acc{i}")
                    for i in range(4)]

            t0 = 0
            for c, W in enumerate(chunks):
                E = W + 6
                t_lo = max(0, t0 - 3)
                t_hi = min(HS, t0 + W + 3)
                col_lo = t_lo - (t0 - 3)
                ncols = t_hi - t_lo

                hand_t = inp.tile([P, E, 2, 3], f32, tag="hand")
                obj_t = inp.tile([P, E, 3], f32, tag="obj")
                probs_t = inp.tile([P, E, 3], f32, tag="probs")

                loads = (
                    (hand_t, hand, 6, nc.sync),
                    (obj_t, obj, 3, nc.scalar),
                    (probs_t, probs, 3, nc.scalar),
                )
                for tile_buf, ten, k, eng in loads:
                    for h in range(2):
                        eng.dma_start(
                            out=tile_buf[h * H:(h + 1) * H, col_lo:col_lo + ncols],
                            in_=dram_ap(ten, (h * HS + t_lo) * k,
                                        [[seq * k, bs_local], [1, ncols * k]]),
                        )
                    if c == 0:
                        eng.dma_start(
                            out=tile_buf[H:P, 0:3],
                            in_=dram_ap(ten, (HS - 3) * k,
                                        [[seq * k, bs_local], [1, 3 * k]]),
                        )
                        nc.vector.memset(tile_buf[0:H, 0:3], 0.0)
                    if c == C - 1:
                        eng.dma_start(
                            out=tile_buf[0:H, W + 3:E],
                            in_=dram_ap(ten, HS * k,
                                        [[seq * k, bs_local], [1, 3 * k]]),
                        )
                        nc.vector.memset(tile_buf[H:P, W + 3:E], 0.0)

                # ---- fused: r[c,t,h] = hand[t,h,c] - obj[t,c]  (fp32 -> bf16) ----
                r_t = work.tile([P, 3, E, 2], bf16, tag="r")
                hand_v = fview(hand_t, [[1, 3], [6, E], [3, 2]])
                obj_v = fview(obj_t, [[1, 3], [3, E], [0, 2]])
                nc.vector.tensor_sub(r_t[:], hand_v, obj_v)

                # ---- sq = r^2 (ACT, contiguous) ----
                sq_t = work.tile([P, 3, E, 2], bf16, tag="sq")
                nc.scalar.activation(sq_t[:], r_t[:], Act.Square)

                # ---- d2 = sum_c sq  (contiguous bf16 2x adds) ----
                d2a_t = work.tile([P, E, 2], bf16, tag="tmpa")
                nc.vector.tensor_add(d2a_t[:], sq_t[:, 0], sq_t[:, 1])
                d2_t = work.tile([P, E, 2], bf16, tag="d2")
                nc.vector.tensor_add(d2_t[:], d2a_t[:], sq_t[:, 2])
                d_t = work.tile([P, E, 2], bf16, tag="d")
                nc.scalar.activation(d_t[:], d2_t[:], Act.Sqrt)

                # ---- probs2 -> bf16 once (strided fp32 read, 2x_2p copy) ----
                probsb_t = work.tile([P, E, 2], bf16, tag="probsb")
                nc.vector.tensor_copy(probsb_t[:], probs_t[:, :, 0:2])

                # ---- contact distance: z1 = probs2 * |d - 0.1| ----
                derr_t = work.tile([P, W, 2], bf16, tag="derr")
                nc.scalar.activation(derr_t[:], d_t[:, 3:3 + W, :], Act.Abs, bias=c_neg01[:])
                z1_t = work.tile([P, W, 2], bf16, tag="z1")
                nc.vector.tensor_mul(z1_t[:], probsb_t[:, 3:3 + W, :], derr_t[:])

                # ---- velocity: z2 = probs2[t] * ||dr|| ----
                dr_t = work.tile([P, 3, W, 2], bf16, tag="dr")
                nc.vector.tensor_sub(dr_t[:], r_t[:, :, 4:4 + W, :], r_t[:, :, 3:3 + W, :])
                dsq_t = work.tile([P, 3, W, 2], bf16, tag="dsq")
                nc.scalar.activation(dsq_t[:], dr_t[:], Act.Square)
                v2a_t = work.tile([P, W, 2], bf16, tag="tmpa")
                nc.vector.tensor_add(v2a_t[:], dsq_t[:, 0], dsq_t[:, 1])
                v2_t = work.tile([P, W, 2], bf16, tag="v2")
                nc.vector.tensor_add(v2_t[:], v2a_t[:], dsq_t[:, 2])
                vd_t = work.tile([P, W, 2], bf16, tag="vd")
                nc.scalar.activation(vd_t[:], v2_t[:], Act.Sqrt)
                if c == C - 1:
                    nc.vector.memset(vd_t[H:P, W - 1:W, :], 0.0)  # j=seq-1 invalid
                z2_t = work.tile([P, W, 2], bf16, tag="z2")
                nc.vector.tensor_mul(z2_t[:], probsb_t[:, 4:4 + W, :], vd_t[:])

                # ---- smoothness ----
                e_t = work.tile([P, E - 1, 2], bf16, tag="e")
                nc.vector.tensor_sub(e_t[:], d_t[:, 1:E, :], d_t[:, 0:E - 1, :])
                sdp_t = work.tile([P, W + 4, 2], bf16, tag="sdp")
                nc.vector.tensor_sub(sdp_t[:], e_t[:, 0:W + 4, :], e_t[:, 1:W + 5, :])
                sd_t = work.tile([P, W + 4, 2], bf16, tag="sd")
                nc.scalar.activation(sd_t[:], sdp_t[:], Act.Abs)
                s2_t = work.tile([P, W + 3, 2], bf16, tag="s2")
                nc.vector.tensor_add(s2_t[:], sd_t[:, 0:W + 3, :], sd_t[:, 1:W + 4, :])
                s4_t = work.tile([P, W + 1, 2], bf16, tag="s4")
                nc.vector.tensor_add(s4_t[:], s2_t[:, 0:W + 1, :], s2_t[:, 2:W + 3, :])
                sm5_t = work.tile([P, W, 2], bf16, tag="sm5")
                nc.vector.tensor_add(sm5_t[:], s4_t[:, 0:W, :], sd_t[:, 4:W + 4, :])

                # ---- first contact via +-1 Sign (exact fp32 threshold) ----
                cb_t = work.tile([P, W + 1, 2], bf16, tag="cb")
                nc.scalar.activation(cb_t[:], probs_t[:, 2:3 + W, 0:2], Act.Sign, bias=c_neg05[:])
                q_t = work.tile([P, W, 2], bf16, tag="q")
                nc.vector.tensor_sub(q_t[:], cb_t[:, 1:W + 1, :], cb_t[:, 0:W, :])
                if c == 0:
                    nc.vector.memset(q_t[0:H, 0:3, :], 0.0)  # t<3 (incl. forced-false t=0)
                if c == C - 1:
                    nc.vector.memset(q_t[H:P, W - 3:W, :], 0.0)  # t >= seq-3
                fc_t = work.tile([P, W, 2], bf16, tag="fc")
                nc.vector.tensor_scalar(
                    out=fc_t[:], in0=q_t[:], scalar1=0.0, scalar2=None, op0=Alu.max)
                z3_t = work.tile([P, W, 2], bf16, tag="z3")
                nc.vector.tensor_mul(z3_t[:], sm5_t[:], fc_t[:])

                # ---- reductions on TensorE: ones^T @ z -> PSUM column sums ----
                for acc, zt in zip(accs, (z1_t, z2_t, z3_t, fc_t)):
                    zv = fview(zt, [[1, 2 * W]])
                    for off in range(0, 2 * W, 512):
                        n = min(512, 2 * W - off)
                        nc.tensor.matmul(
                            out=acc[0:1, 0:n], lhsT=ones[:], rhs=zv[:, off:off + n],
                            start=(c == 0 and off == 0), stop=(c == C - 1 and off + n == 2 * W),
                            skip_group_check=True)

                t0 += W

            # ---- final: reduce each PSUM accumulator row to one scalar ----
            for i, acc in enumerate(accs):
                nc.vector.tensor_reduce(outt[0:1, i:i + 1], acc[:], axis=mybir.AxisListType.X, op=Alu.add)
            nc.sync.dma_start(out=partials.ap(), in_=outt[:])

    nc.compile()
    return nc


_cache = {}


def _get_nc(bs_local, seq, chunks):
    key = (bs_local, seq, chunks)
    if key not in _cache:
        _cache[key] = build_nc(bs_local, seq, chunks)
    return _cache[key]


def combine_partials(parts, bs, seq, training_step):
    """parts: float array [..., 4] of per-core/per-partition partial sums.

    Slots [2] (sm total) and [3] (count) are doubled by the +-1 Sign encoding.
    """
    s = np.asarray(parts, dtype=np.float64).reshape(-1, 4).sum(axis=0)
    l1 = s[0] / (bs * seq * 2)
    l2 = s[1] / (bs * (seq - 1) * 2) if seq > 1 else 0.0
    cnt = s[3] / 2.0
    sm = (s[2] / 2.0 / 5.0) / max(cnt, 1.0) if (seq > 5 and cnt > 0) else 0.0
    ramp = min(1.0, float(training_step) / 1000.0)
    return np.array(ramp * (1.0 * l1 + 0.5 * l2 + 0.3 * sm), dtype=np.float32)


def _run(pred_hand_pos, pred_obj_pos, contact_probs, **spmd_kwargs):
    from concourse.bass_utils import run_bass_kernel_spmd

    hand = np.ascontiguousarray(np.asarray(pred_hand_pos, dtype=np.float32))
    obj = np.ascontiguousarray(np.asarray(pred_obj_pos, dtype=np.float32))
    probs = np.ascontiguousarray(np.asarray(contact_probs, dtype=np.float32))
    bs, seq = hand.shape[:2]
    bs_local = bs // N_CORES
    nc = _get_nc(bs_local, seq, CHUNKS)

    in_maps = []
    for i in range(N_CORES):
        sl = slice(i * bs_local, (i + 1) * bs_local)
        in_maps.append({
            "pred_hand_pos": hand[sl],
            "pred_obj_pos": obj[sl],
            "contact_probs": probs[sl],
        })
    # The axon terminal occasionally reports the exec unit unrecoverable on
    # the first touch after a previous process's teardown; a retry lands on a
    # recovered device.
    last_err = None
    for _ in range(3):
        try:
            res = run_bass_kernel_spmd(
                nc, in_maps, core_ids=list(range(N_CORES)), **spmd_kwargs
            )
            parts = np.stack([res.results[i]["partials"] for i in range(N_CORES)])
            return parts, res
        except Exception as e:  # noqa: BLE001
            last_err = e
    raise last_err


def kernel(pred_hand_pos, pred_obj_pos, contact_probs, training_step):
    bs, seq = np.asarray(pred_hand_pos).shape[:2]
    parts, _ = _run(pred_hand_pos, pred_obj_pos, contact_probs)
    return combine_partials(parts, bs, seq, training_step)


# revision 6
# speedup vs baseline: 2.8154x; 1.0214x over previous
"""ContactAwareLoss Trainium2 kernel.

Strategy: pure data-parallel over batch (512 rows -> 8 cores x 64 rows).
Each core computes four partial sums over its shard:
  [0] sum_{t,h} probs2 * |dist - 0.1|            (contact distance, unnormalized)
  [1] sum_{j,h} probs2[j+1] * ||r[j+1]-r[j]||     (contact velocity, unnormalized)
  [2] 2 * sum first_contact * (5-tap sum of |second diff of dist|)
  [3] 2 * sum first_contact                       (count; x2 from the +-1 Sign encoding)
The host divides by the global element counts / count and applies the ramp.

On-chip layout: partition p = half*64 + b  (sequence halved so 64 batch rows
fill 128 partitions); free dim = time within the half, processed in chunks
with a 3-element halo on both sides.  The first chunk is narrow so the
pipeline-fill DMA is short.  The halo at the half boundary is filled with
real neighbour data via small extra DMAs; the halo at the global sequence
ends is zero-filled and the affected contributions are masked by zeroing
q/vd edge columns.

DMA: one dma_start per tensor per sequence-half so the outermost AP dim is
bs_local (64) -- the descriptor->SDMA-engine assignment follows the outermost
AP dim, so this spreads the load over all 16 engines instead of 2.

Engine split (measured modes):
 - DVE: all tensor_tensor work in bf16 contiguous (2x packed mode); the
   geometry runs in a (c, t, h) layout so channel sums are contiguous adds;
   r is ONE fused fp32 subtract that deinterleaves hand and broadcasts obj
   via a stride-0 AP (1x, but replaces three ops).  fc = max(q,0) uses
   tensor_scalar WITHOUT accum_out (accum forces 1x; no-accum runs 4x).
 - ACT: Square/Sqrt/Abs chains + Sign for the contact threshold (strided
   fp32 inputs run at full ACT speed; exact fp32 compare, +-1 encoding).
 - TensorE (idle otherwise): all four row-sum reductions as ones-vector
   matmuls accumulating into PSUM across chunks (host only needs global
   sums, not per-partition ones).
 - GpSimd: unused (measured ~10x below its cost model).
"""

import numpy as np

BS, SEQ = 512, 4096
N_CORES = 8
CHUNKS = (128, 512, 512, 512, 384)  # sums to 2048 = HS; narrow first chunk


def build_nc(bs_local, seq, chunks):
    import concourse.bass as bass
    import concourse.bacc as bacc
    import concourse.tile as tile
    from concourse import mybir

    f32 = mybir.dt.float32
    bf16 = mybir.dt.bfloat16
    Alu = mybir.AluOpType
    Act = mybir.ActivationFunctionType

    P = 2 * bs_local          # partitions used
    HS = seq // 2             # timesteps per partition row
    assert sum(chunks) == HS
    C = len(chunks)
    H = P // 2

    nc = bacc.Bacc("TRN2", target_bir_lowering=False, debug=False)
    hand = nc.dram_tensor("pred_hand_pos", [bs_local, seq, 2, 3], f32, kind="ExternalInput")
    obj = nc.dram_tensor("pred_obj_pos", [bs_local, seq, 3], f32, kind="ExternalInput")
    probs = nc.dram_tensor("contact_probs", [bs_local, seq, 3], f32, kind="ExternalInput")
    partials = nc.dram_tensor("partials", [P, 4], f32, kind="ExternalOutput")

    def dram_ap(t, offset, dims):
        return bass.AP(tensor=t, offset=offset, ap=[list(d) for d in dims])

    def fview(t, dims):
        """Free-dim view of a tile with explicit [stride, count] dims."""
        return bass.AP(tensor=t.tensor, offset=t[:].offset,
                       ap=[t[:].ap[0]] + [list(d) for d in dims])

    with tile.TileContext(nc) as tc:
        import contextlib
        with contextlib.ExitStack() as ctx:
            inp = ctx.enter_context(tc.tile_pool(name="inp", bufs=2))
            work = ctx.enter_context(tc.tile_pool(name="work", bufs=2))
            singles = ctx.enter_context(tc.tile_pool(name="singles", bufs=1))
            psum = ctx.enter_context(tc.tile_pool(name="psum", bufs=1, space="PSUM"))

            outt = singles.tile([P, 4], f32)
            nc.vector.memset(outt[:], 0.0)
            c_neg01 = singles.tile([P, 1], f32)
            nc.vector.memset(c_neg01[:], -0.1)
            c_neg05 = singles.tile([P, 1], f32)
            nc.vector.memset(c_neg05[:], -0.5)
            ones = singles.tile([P, 1], bf16)
            nc.vector.memset(ones[:], 1.0)

            # PSUM accumulators: one bank each for l1/l2/sm/cnt column sums.
            accs = [psum.tile([1, 512], f32, name=f"acc{i}", tag=f"acc{i}")
                    for i in range(4)]

            t0 = 0
            for c, W in enumerate(chunks):
                E = W + 6
                t_lo = max(0, t0 - 3)
                t_hi = min(HS, t0 + W + 3)
                col_lo = t_lo - (t0 - 3)
                ncols = t_hi - t_lo

                hand_t = inp.tile([P, E, 2, 3], f32, tag="hand")
                obj_t = inp.tile([P, E, 3], f32, tag="obj")
                probs_t = inp.tile([P, E, 3], f32, tag="probs")

                loads = (
                    (hand_t, hand, 6, nc.sync),
                    (obj_t, obj, 3, nc.scalar),
                    (probs_t, probs, 3, nc.scalar),
                )
                for tile_buf, ten, k, eng in loads:
                    for h in range(2):
                        eng.dma_start(
                            out=tile_buf[h * H:(h + 1) * H, col_lo:col_lo + ncols],
                            in_=dram_ap(ten, (h * HS + t_lo) * k,
                                        [[seq * k, bs_local], [1, ncols * k]]),
                        )
                    if c == 0:
                        eng.dma_start(
                            out=tile_buf[H:P, 0:3],
                            in_=dram_ap(ten, (HS - 3) * k,
                                        [[seq * k, bs_local], [1, 3 * k]]),
                        )
                        nc.vector.memset(tile_buf[0:H, 0:3], 0.0)
                    if c == C - 1:
                        eng.dma_start(
                            out=tile_buf[0:H, W + 3:E],
                            in_=dram_ap(ten, HS * k,
                                        [[seq * k, bs_local], [1, 3 * k]]),
                        )
                        nc.vector.memset(tile_buf[H:P, W + 3:E], 0.0)

                # Statement order = per-engine issue order; interleaved so each
                # engine has ready work while the other runs its long ops.

                # ACT: cb first (needs only probs; exact fp32 threshold, +-1)
                cb_t = work.tile([P, W + 1, 2], bf16, tag="cb")
                nc.scalar.activation(cb_t[:], probs_t[:, 2:3 + W, 0:2], Act.Sign, bias=c_neg05[:])

                # DVE: probsb cast + fused r while ACT runs cb
                probsb_t = work.tile([P, E, 2], bf16, tag="probsb")
                nc.vector.tensor_copy(probsb_t[:], probs_t[:, :, 0:2])
                # r[c,t,h] = hand[t,h,c] - obj[t,c]  (fp32 -> bf16, deinterleave
                # + stride-0 obj broadcast in one op)
                r_t = work.tile([P, 3, E, 2], bf16, tag="r")
                hand_v = fview(hand_t, [[1, 3], [6, E], [3, 2]])
                obj_v = fview(obj_t, [[1, 3], [3, E], [0, 2]])
                nc.vector.tensor_sub(r_t[:], hand_v, obj_v)

                # ACT: sq = r^2 while DVE does dr/q/fc
                sq_t = work.tile([P, 3, E, 2], bf16, tag="sq")
                nc.scalar.activation(sq_t[:], r_t[:], Act.Square)

                dr_t = work.tile([P, 3, W, 2], bf16, tag="dr")
                nc.vector.tensor_sub(dr_t[:], r_t[:, :, 4:4 + W, :], r_t[:, :, 3:3 + W, :])
                q_t = work.tile([P, W, 2], bf16, tag="q")
                nc.vector.tensor_sub(q_t[:], cb_t[:, 1:W + 1, :], cb_t[:, 0:W, :])
                if c == 0:
                    nc.vector.memset(q_t[0:H, 0:3, :], 0.0)  # t<3 (incl. forced-false t=0)
                if c == C - 1:
                    nc.vector.memset(q_t[H:P, W - 3:W, :], 0.0)  # t >= seq-3
                fc_t = work.tile([P, W, 2], bf16, tag="fc")
                nc.vector.tensor_scalar(
                    out=fc_t[:], in0=q_t[:], scalar1=0.0, scalar2=None, op0=Alu.max)

                # ACT: dsq = dr^2 (dr just finished)
                dsq_t = work.tile([P, 3, W, 2], bf16, tag="dsq")
                nc.scalar.activation(dsq_t[:], dr_t[:], Act.Square)

                # DVE: d2 channel sums (sq ready by now)
                d2a_t = work.tile([P, E, 2], bf16, tag="tmpa")
                nc.vector.tensor_add(d2a_t[:], sq_t[:, 0], sq_t[:, 1])
                d2_t = work.tile([P, E, 2], bf16, tag="d2")
                nc.vector.tensor_add(d2_t[:], d2a_t[:], sq_t[:, 2])

                # ACT: d = sqrt(d2)
                d_t = work.tile([P, E, 2], bf16, tag="d")
                nc.scalar.activation(d_t[:], d2_t[:], Act.Sqrt)

                # DVE: v2 channel sums (dsq ready)
                v2a_t = work.tile([P, W, 2], bf16, tag="tmpa")
                nc.vector.tensor_add(v2a_t[:], dsq_t[:, 0], dsq_t[:, 1])
                v2_t = work.tile([P, W, 2], bf16, tag="v2")
                nc.vector.tensor_add(v2_t[:], v2a_t[:], dsq_t[:, 2])

                # ACT: vd = sqrt(v2); derr = |d - 0.1|
                vd_t = work.tile([P, W, 2], bf16, tag="vd")
                nc.scalar.activation(vd_t[:], v2_t[:], Act.Sqrt)
                # (no vd edge mask needed: the zero-filled probs halo already
                #  zeroes the j=seq-1 product below)
                derr_t = work.tile([P, W, 2], bf16, tag="derr")
                nc.scalar.activation(derr_t[:], d_t[:, 3:3 + W, :], Act.Abs, bias=c_neg01[:])

                # DVE: smoothness first diffs (d ready)
                e_t = work.tile([P, E - 1, 2], bf16, tag="e")
                nc.vector.tensor_sub(e_t[:], d_t[:, 1:E, :], d_t[:, 0:E - 1, :])
                sdp_t = work.tile([P, W + 4, 2], bf16, tag="sdp")
                nc.vector.tensor_sub(sdp_t[:], e_t[:, 0:W + 4, :], e_t[:, 1:W + 5, :])

                # ACT: sd = |sdp|
                sd_t = work.tile([P, W + 4, 2], bf16, tag="sd")
                nc.scalar.activation(sd_t[:], sdp_t[:], Act.Abs)

                # DVE: weight products + 5-tap movsum
                z2_t = work.tile([P, W, 2], bf16, tag="z2")
                nc.vector.tensor_mul(z2_t[:], probsb_t[:, 4:4 + W, :], vd_t[:])
                z1_t = work.tile([P, W, 2], bf16, tag="z1")
                nc.vector.tensor_mul(z1_t[:], probsb_t[:, 3:3 + W, :], derr_t[:])
                s2_t = work.tile([P, W + 3, 2], bf16, tag="s2")
                nc.vector.tensor_add(s2_t[:], sd_t[:, 0:W + 3, :], sd_t[:, 1:W + 4, :])
                s4_t = work.tile([P, W + 1, 2], bf16, tag="s4")
                nc.vector.tensor_add(s4_t[:], s2_t[:, 0:W + 1, :], s2_t[:, 2:W + 3, :])
                sm5_t = work.tile([P, W, 2], bf16, tag="sm5")
                nc.vector.tensor_add(sm5_t[:], s4_t[:, 0:W, :], sd_t[:, 4:W + 4, :])
                z3_t = work.tile([P, W, 2], bf16, tag="z3")
                nc.vector.tensor_mul(z3_t[:], sm5_t[:], fc_t[:])

                # ---- reductions on TensorE: ones^T @ z -> PSUM column sums ----
                for acc, zt in zip(accs, (z1_t, z2_t, z3_t, fc_t)):
                    zv = fview(zt, [[1, 2 * W]])
                    for off in range(0, 2 * W, 512):
                        n = min(512, 2 * W - off)
                        nc.tensor.matmul(
                            out=acc[0:1, 0:n], lhsT=ones[:], rhs=zv[:, off:off + n],
                            start=(c == 0 and off == 0), stop=(c == C - 1 and off + n == 2 * W),
                            skip_group_check=True)

                t0 += W

            # ---- final: reduce each PSUM accumulator row to one scalar ----
            for i, acc in enumerate(accs):
                nc.vector.tensor_reduce(outt[0:1, i:i + 1], acc[:], axis=mybir.AxisListType.X, op=Alu.add)
            nc.sync.dma_start(out=partials.ap(), in_=outt[:])

    nc.compile()
    return nc


_cache = {}


def _get_nc(bs_local, seq, chunks):
    key = (bs_local, seq, chunks)
    if key not in _cache:
        _cache[key] = build_nc(bs_local, seq, chunks)
    return _cache[key]


def combine_partials(parts, bs, seq, training_step):
    """parts: float array [..., 4] of per-core/per-partition partial sums.

    Slots [2] (sm total) and [3] (count) are doubled by the +-1 Sign encoding.
    """
    s = np.asarray(parts, dtype=np.float64).reshape(-1, 4).sum(axis=0)
    l1 = s[0] / (bs * seq * 2)
    l2 = s[1] / (bs * (seq - 1) * 2) if seq > 1 else 0.0
    cnt = s[3] / 2.0
    sm = (s[2] / 2.0 / 5.0) / max(cnt, 1.0) if (seq > 5 and cnt > 0) else 0.0
    ramp = min(1.0, float(training_step) / 1000.0)
    return np.array(ramp * (1.0 * l1 + 0.5 * l2 + 0.3 * sm), dtype=np.float32)


def _run(pred_hand_pos, pred_obj_pos, contact_probs, **spmd_kwargs):
    from concourse.bass_utils import run_bass_kernel_spmd

    hand = np.ascontiguousarray(np.asarray(pred_hand_pos, dtype=np.float32))
    obj = np.ascontiguousarray(np.asarray(pred_obj_pos, dtype=np.float32))
    probs = np.ascontiguousarray(np.asarray(contact_probs, dtype=np.float32))
    bs, seq = hand.shape[:2]
    bs_local = bs // N_CORES
    nc = _get_nc(bs_local, seq, CHUNKS)

    in_maps = []
    for i in range(N_CORES):
        sl = slice(i * bs_local, (i + 1) * bs_local)
        in_maps.append({
            "pred_hand_pos": hand[sl],
            "pred_obj_pos": obj[sl],
            "contact_probs": probs[sl],
        })
    # The axon terminal occasionally reports the exec unit unrecoverable on
    # the first touch after a previous process's teardown; a retry lands on a
    # recovered device.
    last_err = None
    for _ in range(3):
        try:
            res = run_bass_kernel_spmd(
                nc, in_maps, core_ids=list(range(N_CORES)), **spmd_kwargs
            )
            parts = np.stack([res.results[i]["partials"] for i in range(N_CORES)])
            return parts, res
        except Exception as e:  # noqa: BLE001
            last_err = e
    raise last_err


def kernel(pred_hand_pos, pred_obj_pos, contact_probs, training_step):
    bs, seq = np.asarray(pred_hand_pos).shape[:2]
    parts, _ = _run(pred_hand_pos, pred_obj_pos, contact_probs)
    return combine_partials(parts, bs, seq, training_step)
